# revision 3
# baseline (speedup 1.0000x reference)
"""GAT encoder (3-layer) on 8 Trainium2 NeuronCores — v2.

Sharding: nodes partitioned across cores; edges partitioned by destination.
Per layer: each core computes [Wh | alpha_src] for its OWN nodes (bf16,
node-major 512B rows), AllGathers the shards; the gathered tensor IS the
edge-gather table. Edge phase: edges sorted by dst window (128 nodes);
per 128-edge tile a fused DVE op builds lhsT[e,d] = (iota==ld)*p with
p = exp(leakyrelu(as_src + ad_dst)); TensorE accumulates
out[dst, 0:129] = sum_e p*[Wh_src | 1] in PSUM per window (numerator +
denominator in one matmul). No DMA scatter. alpha_d per edge comes from a
per-layer GpSimd indirect_copy out of a broadcast ad_rep table.
"""

import math
import numpy as np

# ---------------- constants (hardcoded problem shape) ----------------
N = 50000
F = 128
G = 64
NCORES = 8
NLOC = 6272                   # 49*128 nodes per core
NPAD = NLOC * NCORES          # 50176
NW = NLOC // 128              # 49 windows per core
RW = 256                      # table row width (bf16) -> 512B
BKA = 31360                   # bank A rows = cores 0-4 (int16-safe)
GCH = 32                      # tiles per dma_gather chunk (4096 tokens)
NEG_SLOPE = 0.2
BIG_NEG = -1.0e9
EPS = 1.0e-16
AW = 132                      # ad_rep cols per window (128 + 4 sentinel)
ADW = NW * AW                 # ad_rep width (window-blocked)
TC = 32                       # tiles per indirect_copy call (16*TC <= 512)


# ---------------- host-side preprocessing ----------------

def _balance_perm(src, dst):
    """Per-core permutation of local slots balancing per-window edge counts.

    Returns gperm[NPAD]: global node id -> permuted global table row.
    """
    src_core = src // NLOC
    gperm = np.zeros(NPAD, np.int64)
    for r in range(NCORES):
        lo = r * NLOC
        m = (dst >= lo) & (dst < lo + NLOC)
        ld = dst[m] - lo
        inA = src_core[m] <= 4
        dA = np.bincount(ld[inA], minlength=NLOC).astype(np.float64)
        dB = np.bincount(ld[~inA], minlength=NLOC).astype(np.float64)
        tgtA = max(dA.sum() / NW, 1.0)
        tgtB = max(dB.sum() / NW, 1.0)
        order = np.argsort(-(dA + dB), kind="stable")
        eA = np.zeros(NW)
        eB = np.zeros(NW)
        used = np.zeros(NW, np.int64)
        perm = np.zeros(NLOC, np.int64)
        for j in order:
            cost = np.maximum((eA + dA[j]) / tgtA, (eB + dB[j]) / tgtB)
            cost[used >= 128] = np.inf
            w = int(np.argmin(cost))
            perm[j] = w * 128 + used[w]
            used[w] += 1
            eA[w] += dA[j]
            eB[w] += dB[j]
        gperm[lo:lo + NLOC] = lo + perm
    return gperm


def _build_schedule(src, dst):
    """Per-core edge tiling. Returns (sched, per_core_idx).

    sched: list over windows w of (tA[w], tB[w]) tile counts (uniform
    across cores). per_core_idx[r]: dict with gidxA, gidxB (int16 token
    streams, bank-relative row ids), ld (float32 [128, NT]),
    adidx (uint16 [128, NT]).
    """
    gperm = _balance_perm(src, dst)
    psrc = gperm[src]
    pdst = gperm[dst]
    # per (core, window, bank) edge lists
    lists = [[[None, None] for _ in range(NW)] for _ in range(NCORES)]
    for r in range(NCORES):
        lo, hi = r * NLOC, (r + 1) * NLOC
        m = (pdst >= lo) & (pdst < hi)
        gs = psrc[m]                     # global src table row
        ld = pdst[m] - lo                # local dst (permuted)
        w = ld // 128
        grow = gs                        # global table row == node id
        bankB = grow >= BKA
        for wi in range(NW):
            mw = w == wi
            ga, la = grow[mw & ~bankB], ld[mw & ~bankB]
            gb, lb = grow[mw & bankB], ld[mw & bankB]
            lists[r][wi][0] = (ga, la)
            lists[r][wi][1] = (gb - BKA, lb)

    tA = np.zeros(NW, np.int64)
    tB = np.zeros(NW, np.int64)
    for wi in range(NW):
        for r in range(NCORES):
            tA[wi] = max(tA[wi], (len(lists[r][wi][0][0]) + 127) // 128)
            tB[wi] = max(tB[wi], (len(lists[r][wi][1][0]) + 127) // 128)

    NTA, NTB = int(tA.sum()), int(tB.sum())
    NT = NTA + NTB

    per_core = []
    for r in range(NCORES):
        rowsA = np.zeros((NTA, 128), np.int64)   # bank-relative row ids
        rowsB = np.zeros((NTB, 128), np.int64)
        # all per-tile arrays indexed by STREAM position:
        # bank A tiles at [0, NTA), bank B tiles at [NTA, NT)
        ld_all = np.zeros((NT, 128), np.int64)   # 0..127 within window
        valid = np.zeros((NT, 128), bool)
        win_of = np.zeros(NT, np.int64)
        pa = pb = 0
        for wi in range(NW):
            for bank in (0, 1):
                gs, ldl = lists[r][wi][bank]
                ntile = int(tA[wi] if bank == 0 else tB[wi])
                ne = len(gs)
                for j in range(ntile):
                    s = slice(j * 128, min((j + 1) * 128, ne))
                    nn = s.stop - s.start if s.stop > s.start else 0
                    if bank == 0:
                        scol = pa
                        if nn > 0:
                            rowsA[pa, :nn] = gs[s]
                        pa += 1
                    else:
                        scol = NTA + pb
                        if nn > 0:
                            rowsB[pb, :nn] = gs[s]
                        pb += 1
                    if nn > 0:
                        ld_all[scol, :nn] = ldl[s] - wi * 128
                        valid[scol, :nn] = True
                    win_of[scol] = wi
        assert pa == NTA and pb == NTB

        def tok_stream(rows):
            # token t -> idx[t%16, t//16]; replicated across the 8
            # 16-partition groups (hw: each Q7 core reads its own 16)
            ntok = rows.shape[0] * 128
            flat = rows.reshape(-1)
            out = np.zeros((16, (ntok + 15) // 16), np.int16)
            tt = np.arange(ntok)
            out[tt % 16, tt // 16] = flat.astype(np.int16)
            return np.tile(out, (8, 1))

        adval = np.where(valid, ld_all + win_of[:, None] * AW,
                         win_of[:, None] * AW + 128).astype(np.uint16).T
        # indirect_copy group-stream layout: per 16-partition group g, the
        # index stream position i=(k*TCc+j) lives at aidx[16g+i%16, c0+i//16]
        aidx = np.zeros((128, NT), np.uint16)
        ic_wspan = []
        for c0 in range(0, NT, TC):
            tcc = min(TC, NT - c0)
            wlo = int(win_of[c0:c0 + tcc].min())
            whi = int(win_of[c0:c0 + tcc].max())
            ic_wspan.append((wlo, whi))
            i = np.arange(16 * tcc)
            k, j = i // tcc, i % tcc
            for g in range(8):
                aidx[16 * g + i % 16, c0 + i // 16] = \
                    adval[16 * g + k, c0 + j] - wlo * AW
        per_core.append({
            "ic_wspan": ic_wspan,
            "gidxA": tok_stream(rowsA),
            "gidxB": tok_stream(rowsB),
            "ld": np.ascontiguousarray(ld_all.T).astype(np.float32),
            "adidx": aidx,
            "adval": adval,
            "valid": valid,
            "win_of": win_of,
        })
    # unify ic window spans across cores (program is shared)
    nic = len(per_core[0]["ic_wspan"])
    spans = []
    for c in range(nic):
        wlo = min(pc["ic_wspan"][c][0] for pc in per_core)
        whi = max(pc["ic_wspan"][c][1] for pc in per_core)
        spans.append((wlo, whi))
    for pc in per_core:
        # re-relativize aidx to the unified wlo
        for c, (wlo_u, _) in enumerate(spans):
            wlo_c = pc["ic_wspan"][c][0]
            c0 = c * TC
            tcc = min(TC, (NTA + NTB) - c0)
            pc["adidx"][:, c0:c0 + tcc] += np.uint16((wlo_c - wlo_u) * AW)
    return (tA, tB, NTA, NTB, spans), per_core, gperm


def _to_bf16(x):
    import ml_dtypes
    return np.asarray(x, np.float32).astype(ml_dtypes.bfloat16)


def _prep_inputs(x, edge_index, batch, Ws, asrcs, adsts, bs):
    src = np.concatenate([np.asarray(edge_index[0], np.int64),
                          np.arange(N, dtype=np.int64)])
    dst = np.concatenate([np.asarray(edge_index[1], np.int64),
                          np.arange(N, dtype=np.int64)])

    sched, per_core, gperm = _build_schedule(src, dst)
    ginv = np.argsort(gperm)             # table row -> original node id
    tA, tB, NTA, NTB, ic_spans = sched
    NT = NTA + NTB

    # weights: rhs [F, RW]: [:,0:128]=W, [:,128]=W@a_src, rest 0
    w_rhs = np.zeros((3, F, RW), np.float32)
    wad = np.zeros((3, F, 1), np.float32)
    for k in range(3):
        w_rhs[k, :, :F] = Ws[k]
        w_rhs[k, :, F] = Ws[k] @ asrcs[k]
        wad[k, :, 0] = Ws[k] @ adsts[k]

    b_rep = np.zeros((3, 128, F), np.float32)
    for k in range(3):
        b_rep[k] = np.tile(bs[k][None, :], (128, 1))

    iota = np.tile(np.arange(128, dtype=np.float32)[None, :], (128, 1))

    batch64 = np.asarray(batch, np.int64)
    phot = np.zeros((NCORES, NW * 128, G), np.float32)
    for r in range(NCORES):
        nodes = ginv[r * NLOC + np.arange(NLOC)]   # original node per slot
        vmask = nodes < N
        gsel = batch64[np.minimum(nodes, N - 1)]
        ph = np.zeros((NLOC, G), np.float32)
        ph[np.arange(NLOC)[vmask], gsel[vmask]] = 1.0
        phot[r] = ph
    counts = np.bincount(batch64, minlength=G).astype(np.float32)

    xT = np.asarray(x, np.float32).T  # [F, N]
    xpad = np.zeros((F, NPAD), np.float32)
    xpad[:, :N] = xT
    in_maps = []
    for r in range(NCORES):
        lo = r * NLOC
        xT_own = np.ascontiguousarray(
            xpad[:, ginv[lo:lo + NLOC]])           # permuted slots
        pc = per_core[r]
        in_maps.append({
            "xT_own": _to_bf16(xT_own),
            "w_rhs": _to_bf16(w_rhs),
            "wad": _to_bf16(wad),
            "b_rep": b_rep,
            "iota": _to_bf16(iota),
            "gidx": np.concatenate([pc["gidxA"], pc["gidxB"]], axis=1),
            "ld_all": pc["ld"],
            "adidx": pc["adidx"],
            "phot": _to_bf16(phot[r]),
        })
    return in_maps, sched, per_core, counts


# ---------------- numpy emulation of the device program ----------------

def _emulate_full(in_maps, sched, counts, per_core_g=None):
    import ml_dtypes
    bf = ml_dtypes.bfloat16
    tA, tB, NTA, NTB, ic_spans = sched
    NT = NTA + NTB
    hT = [np.asarray(im["xT_own"], np.float32) for im in in_maps]
    pool = np.zeros((G, F), np.float32)
    for k in range(3):
        # boundary: per-core Wh | as, then all-gather
        table = np.zeros((NCORES, NLOC, RW), np.float32)
        ad_rep = []
        for r in range(NCORES):
            w = np.asarray(in_maps[r]["w_rhs"][k], np.float32)
            wh = (hT[r].T @ w).astype(bf).astype(np.float32)  # [NLOC, RW]
            wh[:, F + 1] = 1.0
            table[r] = wh
            ad = (np.asarray(in_maps[r]["wad"][k], np.float32)[:, 0][None, :]
                  @ hT[r])[0].astype(bf).astype(np.float32)    # [NLOC]
            adr = np.full(ADW, BIG_NEG, np.float32)
            adr2 = adr.reshape(NW, AW)
            adr2[:, 0:128] = ad.reshape(NW, 128)
            ad_rep.append(adr)
        tab_flat = table.reshape(NCORES * NLOC, RW)
        new_hT = []
        for r in range(NCORES):
            im = in_maps[r]
            gidx = im["gidx"]
            ld_all = im["ld_all"]          # [128, NT]

            # reconstruct token streams
            def toks(gi, ntok):
                tt = np.arange(ntok)
                return gi[tt % 16, tt // 16].astype(np.int64)
            rowsA = toks(gidx[:, :NTA * 8], NTA * 128)
            rowsB = toks(gidx[:, NTA * 8:], NTB * 128)
            gbufA = tab_flat[rowsA].reshape(NTA, 128, RW)
            gbufB = tab_flat[rowsB + BKA].reshape(NTB, 128, RW)
            # ad per token (group-stream ic is exact gather of adval)
            adt = ad_rep[r][per_core_g[r]["adval"]]   # [128, NT]
            num_all = np.zeros((NW, 128, F + 2), np.float32)
            pa = pb = 0
            for wi in range(NW):
                acc = np.zeros((128, F + 2), np.float32)
                for bank in (0, 1):
                    ntile = int(tA[wi] if bank == 0 else tB[wi])
                    for j in range(ntile):
                        if bank == 0:
                            gb = gbufA[pa]
                            scol = pa
                            pa += 1
                        else:
                            gb = gbufB[pb]
                            scol = NTA + pb
                            pb += 1
                        as_tok = gb[:, F]
                        e = as_tok + adt[:, scol]
                        e = np.maximum(e, NEG_SLOPE * e)
                        p = np.exp(e).astype(np.float32)
                        lhsT = (im["iota"].astype(np.float32)
                                == ld_all[:, scol][:, None]).astype(np.float32)
                        lhsT = (lhsT * p[:, None]).astype(bf).astype(np.float32)
                        acc += lhsT.T @ gb[:, 0:F + 2]
                num_all[wi] = acc
            den = num_all[:, :, F + 1] + EPS
            h1 = num_all[:, :, :F] / den[:, :, None] \
                + np.asarray(im["b_rep"][k][0])[None, None, :]
            h = np.where(h1 > 0, h1, np.exp(np.minimum(h1, 0)) - 1)
            h = h.reshape(NLOC, F).astype(bf).astype(np.float32)
            if k < 2:
                new_hT.append(h.T)
            else:
                ph = np.asarray(im["phot"], np.float32)
                pool += ph.T @ h
        if k < 2:
            hT = new_hT
    return (pool / np.maximum(counts, 1.0)[:, None]).astype(np.float32)


# ---------------- bass program ----------------

def _build_program(sched, features=("gather", "ic", "cc"), repeat=1,
                   debug_dump=False):
    import concourse.bacc as bacc
    import concourse.mybir as mybir
    import concourse.tile as tile
    from concourse import masks
    features = set(features)

    f32 = mybir.dt.float32
    bf16 = mybir.dt.bfloat16
    i16 = mybir.dt.int16
    u16 = mybir.dt.uint16
    AF = mybir.ActivationFunctionType
    ALU = mybir.AluOpType
    AX = mybir.AxisListType

    tA, tB, NTA, NTB, ic_spans = sched
    NT = NTA + NTB
    NCA = (NTA + GCH - 1) // GCH      # gather chunks bank A
    NCB = (NTB + GCH - 1) // GCH

    nc = bacc.Bacc("TRN2", target_bir_lowering=False, debug=False,
                   num_devices=NCORES)

    # --- dram I/O ---
    xT_own_d = nc.dram_tensor("xT_own", [F, NLOC], bf16, kind="ExternalInput")
    w_rhs_d = nc.dram_tensor("w_rhs", [3, F, RW], bf16, kind="ExternalInput")
    wad_d = nc.dram_tensor("wad", [3, F, 1], bf16, kind="ExternalInput")
    b_rep_d = nc.dram_tensor("b_rep", [3, 128, F], f32, kind="ExternalInput")
    iota_d = nc.dram_tensor("iota", [128, 128], bf16, kind="ExternalInput")
    gidx_d = nc.dram_tensor("gidx", [128, (NTA + NTB) * 8], i16,
                            kind="ExternalInput")
    ld_d = nc.dram_tensor("ld_all", [128, NT], f32, kind="ExternalInput")
    adidx_d = nc.dram_tensor("adidx", [128, NT], u16, kind="ExternalInput")
    phot_d = nc.dram_tensor("phot", [NW * 128, G], bf16, kind="ExternalInput")
    pool_out = nc.dram_tensor("pool_part", [G, F], f32, kind="ExternalOutput")
    NT_ = NTA + NTB
    if debug_dump:
        dump_adt = nc.dram_tensor("dump_adt", [128, NT_], bf16,
                                  kind="ExternalOutput")
        dump_adrep = nc.dram_tensor("dump_adrep", [128, ADW], bf16,
                                    kind="ExternalOutput")
        dump_gb0 = nc.dram_tensor("dump_gb0", [128, GCH, RW], bf16,
                                  kind="ExternalOutput")
        dump_num = nc.dram_tensor("dump_num", [128, NW, F + 2], f32,
                                  kind="ExternalOutput")
        dump_hT = nc.dram_tensor("dump_hT", [F, NLOC], bf16,
                                 kind="ExternalOutput")
        dump_cc = nc.dram_tensor("dump_cc", [256, RW], bf16,
                                 kind="ExternalOutput")
        dump_lh = nc.dram_tensor("dump_lh", [16, 128, 128], bf16,
                                 kind="ExternalOutput")
        dump_pc = nc.dram_tensor("dump_pc", [128, GCH], f32,
                                 kind="ExternalOutput")

    # --- internal dram (separate cc buffers per layer) ---
    cc_ins = [nc.dram_tensor(f"cc_in{k}", [NLOC, RW], bf16, kind="Internal")
              for k in range(3)]
    cc_outs = [nc.dram_tensor(f"cc_out{k}", [NCORES, NLOC, RW], bf16,
                              kind="Internal", addr_space="Shared")
               for k in range(3)]

    with tile.TileContext(nc) as tc:
        with (
            tc.tile_pool(name="persist", bufs=1) as persist,
            tc.tile_pool(name="wh", bufs=3) as wh_pool,
            tc.tile_pool(name="gba", bufs=2) as gba_pool,
            tc.tile_pool(name="gbb", bufs=2) as gbb_pool,
            tc.tile_pool(name="pcol", bufs=2) as pcol_pool,
            tc.tile_pool(name="lhs", bufs=4) as lhs_pool,
            tc.tile_pool(name="post", bufs=1) as post_pool,
            tc.tile_pool(name="blk", bufs=2) as blk_pool,
            tc.tile_pool(name="ic", bufs=2) as ic_pool,
            tc.tile_pool(name="psw", bufs=2, space="PSUM") as psw_pool,
            tc.tile_pool(name="pse", bufs=2, space="PSUM") as pse_pool,
            tc.tile_pool(name="psv", bufs=1, space="PSUM") as psv_pool,
            tc.tile_pool(name="pst", bufs=1, space="PSUM") as pst_pool,
            tc.tile_pool(name="psp", bufs=1, space="PSUM") as psp_pool,
        ):
            # persistent tiles
            hT = persist.tile([F, NLOC], bf16, tag="hT")
            iota_sb = persist.tile([128, 128], bf16, tag="iota")
            gidx_sb = persist.tile([128, NT * 8], i16, tag="gidx")
            ld_sb = persist.tile([128, NT], f32, tag="ld")
            adidx_sb = persist.tile([128, NT], u16, tag="adidx")
            adt_all = persist.tile([128, NT], bf16, tag="adt")
            ad_rep = persist.tile([128, ADW], bf16, tag="ad_rep")
            ones_col = persist.tile([1, 128], bf16, tag="ones")
            phot_sb = persist.tile([128, NW, G], bf16, tag="phot")
            num_all = persist.tile([128, NW, F + 2], f32, tag="num")
            h_all = persist.tile([128, NW, F], bf16, tag="h_all")
            b_sb = persist.tile([128, F], f32, tag="b_sb")
            w_sb = persist.tile([F, RW], bf16, tag="w_sb")
            wad_sb = persist.tile([F, 1], bf16, tag="wad_sb")
            identity = persist.tile([128, 128], bf16, tag="ident")
            if debug_dump:
                lh_stage = persist.tile([128, 16, 128], bf16, tag="lhst")
                pc_stage = persist.tile([128, GCH], f32, tag="pcst")

            masks.make_identity(nc, identity[:])
            nc.gpsimd.memset(ones_col[:], 1.0)
            nc.vector.memset(
                ad_rep[:].rearrange("p (w a) -> p w a", a=AW)[:, :, 128:AW],
                BIG_NEG)
            nc.sync.dma_start(iota_sb[:], iota_d.ap())
            nc.sync.dma_start(gidx_sb[:], gidx_d.ap())
            nc.sync.dma_start(ld_sb[:], ld_d.ap())
            nc.sync.dma_start(adidx_sb[:], adidx_d.ap())
            nc.sync.dma_start(
                phot_sb[:],
                phot_d.ap().rearrange("(w p) g -> p w g", p=128))

            for rep in range(repeat):
              nc.sync.dma_start(hT[:], xT_own_d.ap())
              for k in range(3):
                  nc.sync.dma_start(w_sb[:], w_rhs_d.ap()[k])
                  nc.sync.dma_start(wad_sb[:], wad_d.ap()[k])
                  nc.sync.dma_start(b_sb[:], b_rep_d.ap()[k])

                  # ---- boundary: own Wh|as rows -> cc_in; ad_rep ----
                  for j in ([] if "noboundary" in features else range(NW)):
                      ps = psw_pool.tile([128, RW], f32, tag="ps_wh")
                      nc.tensor.matmul(ps[:], hT[:, 128 * j:128 * (j + 1)],
                                       w_sb[:], start=True, stop=True)
                      st = wh_pool.tile([128, RW], bf16, tag="wh_st")
                      nc.scalar.activation(st[:], ps[:], AF.Copy)
                      nc.vector.memset(st[:, F + 1:F + 2], 1.0)
                      nc.scalar.dma_start(
                          cc_ins[k].ap()[128 * j:128 * (j + 1)], st[:])

                  # ad matvec -> stage row -> broadcast into ad_rep
                  pos = NLOC if "noboundary" in features else 0
                  while pos < NLOC:
                      sz = min(512, NLOC - pos)
                      pr = psv_pool.tile([1, 512], f32, tag="ps_adr")
                      nc.tensor.matmul(pr[:, 0:sz], wad_sb[:],
                                       hT[:, pos:pos + sz],
                                       start=True, stop=True)
                      rstage = wh_pool.tile([1, 512], bf16, tag="ad_st")
                      nc.vector.tensor_copy(rstage[:, 0:sz], pr[:, 0:sz])
                      pb = psv_pool.tile([128, 512], f32, tag="ps_adb")
                      nc.tensor.matmul(pb[:, 0:sz], ones_col[:],
                                       rstage[:, 0:sz],
                                       start=True, stop=True)
                      w0b = pos // 128
                      nwb = sz // 128
                      nc.vector.tensor_copy(
                          ad_rep[:].rearrange("p (w a) -> p w a", a=AW)
                          [:, w0b:w0b + nwb, 0:128],
                          pb[:, 0:sz].rearrange("p (w f) -> p w f", f=128))
                      pos += sz

                  # ---- collective ----
                  if "cc" in features:
                      ncc = 2 if "cc2" in features else 1
                      for _ in range(ncc):
                          nc.gpsimd.collective_compute(
                              "AllGather", mybir.AluOpType.bypass,
                              replica_groups=[list(range(NCORES))],
                              ins=[cc_ins[k].ap().opt()],
                              outs=[cc_outs[k].ap().opt()])
                  elif "localcc" in features:
                      for rr in range(NCORES):
                          nc.sync.dma_start(cc_outs[k].ap()[rr],
                                            cc_ins[k].ap()[:])
                  else:
                      for rr in range(NCORES):
                          nc.sync.dma_start(cc_outs[k].ap()[rr],
                                            cc_ins[k].ap()[:])

                  # ---- per-edge ad via chunked indirect copies ----
                  if "ic" in features:
                      for ci_, c0 in enumerate(range(0, NT, TC)):
                          tcc = min(TC, NT - c0)
                          wlo, whi = ic_spans[ci_]
                          ico = ic_pool.tile([128, 16 * TC], bf16, tag="ico")
                          nc.gpsimd.indirect_copy(
                              ico[:, 0:16 * tcc],
                              ad_rep[:, wlo * AW:(whi + 1) * AW],
                              adidx_sb[:, c0:c0 + tcc], True)
                          src_ap = ico[:, 0:16 * tcc].rearrange(
                              "(g o) (kk j) -> g o kk j",
                              g=8, o=16, kk=16, j=tcc)[:, 0]
                          nc.sync.dma_start(adt_all[:, c0:c0 + tcc], src_ap)
                  else:
                      nc.gpsimd.memset(adt_all[:], 0.0)

                  # ---- edge phase ----
                  tab = cc_outs[k].ap().rearrange("c n w -> (c n) w")
                  bankA = tab[0:BKA]
                  bankB = tab[BKA:NPAD]

                  gbufsA, gbufsB = {}, {}
                  pcolsA, pcolsB = {}, {}

                  def issue_chunk(bank, ci):
                      n0 = ci * GCH
                      ntile = min(GCH, (NTA if bank == 0 else NTB) - n0)
                      ntok = ntile * 128
                      pool_ = gba_pool if bank == 0 else gbb_pool
                      gb = pool_.tile([128, GCH, RW], bf16, tag=f"gb{bank}")
                      coff = (n0 + (0 if bank == 0 else NTA)) * 8
                      if "gather" in features:
                          nc.gpsimd.dma_gather(
                              gb[:, 0:ntile], bankA if bank == 0 else bankB,
                              gidx_sb[:, coff:coff + ntile * 8],
                              ntok, ntok, RW, single_packet=False)
                      else:
                          nc.vector.memset(gb[:, 0:ntile], 0.0)
                      # p for this chunk
                      toff = n0 + (0 if bank == 0 else NTA)
                      pc = pcol_pool.tile([128, GCH], f32, tag=f"pc{bank}")
                      e = pcol_pool.tile([128, GCH], f32, tag=f"e{bank}")
                      nc.vector.tensor_tensor(
                          e[:, 0:ntile], gb[:, 0:ntile, F],
                          adt_all[:, toff:toff + ntile], ALU.add)
                      nc.vector.scalar_tensor_tensor(
                          e[:, 0:ntile], e[:, 0:ntile], NEG_SLOPE,
                          e[:, 0:ntile], ALU.mult, ALU.max)
                      nc.scalar.activation(pc[:, 0:ntile], e[:, 0:ntile],
                                           AF.Exp)
                      if bank == 0:
                          gbufsA[ci] = gb
                          pcolsA[ci] = pc
                          if debug_dump and k == 0 and ci == 0:
                              nc.sync.dma_start(dump_gb0.ap()[:], gb[:])
                      else:
                          gbufsB[ci] = gb
                          pcolsB[ci] = pc

                  # prefetch first chunks
                  issue_chunk(0, 0)
                  if NCB > 0:
                      issue_chunk(1, 0)

                  pa = pb_ = 0
                  t = 0
                  for wi in range(NW):
                      ntl = int(tA[wi] + tB[wi])
                      acc = pse_pool.tile([128, F + 2], f32, tag="acc")
                      ti = 0
                      for bank in (0, 1):
                          ntile = int(tA[wi] if bank == 0 else tB[wi])
                          for j in range(ntile):
                              if bank == 0:
                                  ci, sl = pa // GCH, pa % GCH
                                  if (sl == 0 and ci + 1 < NCA
                                          and ci + 1 not in gbufsA):
                                      issue_chunk(0, ci + 1)
                                  gb, pc = gbufsA[ci], pcolsA[ci]
                                  scol = pa
                                  pa += 1
                              else:
                                  ci, sl = pb_ // GCH, pb_ % GCH
                                  if (sl == 0 and ci + 1 < NCB
                                          and ci + 1 not in gbufsB):
                                      issue_chunk(1, ci + 1)
                                  gb, pc = gbufsB[ci], pcolsB[ci]
                                  scol = NTA + pb_
                                  pb_ += 1
                              if "nolh" in features:
                                  ti += 1
                                  continue
                              lh = lhs_pool.tile([128, 128], bf16, tag="lh")
                              nc.vector.tensor_scalar(
                                  lh[:], iota_sb[:],
                                  ld_sb[:, scol:scol + 1],
                                  pc[:, sl:sl + 1], ALU.is_equal, ALU.mult)
                              if debug_dump and k == 0 and t < 16:
                                  nc.vector.tensor_copy(
                                      lh_stage[:, t], lh[:])
                              if debug_dump and k == 0 and t == 0:
                                  nc.vector.tensor_copy(pc_stage[:], pc[:])
                              nc.tensor.matmul(acc[:], lh[:],
                                               gb[:, sl, 0:F + 2],
                                               start=(ti == 0),
                                               stop=(ti == ntl - 1))
                              t += 1
                              ti += 1
                      if "nolh" not in features:
                          nc.scalar.activation(num_all[:, wi], acc[:],
                                               AF.Copy)

                  # ---- post: h = ELU(num/den + b) ----
                  # ELU(x) = max(exp(min(x,0)) - 1, x)
                  if "nopost" in features:
                      continue
                  den = post_pool.tile([128, NW], f32, tag="den")
                  nc.vector.tensor_scalar_add(den[:], num_all[:, :, F + 1], EPS)
                  denr = post_pool.tile([128, NW], f32, tag="denr")
                  nc.vector.reciprocal(denr[:], den[:])
                  WB = 7
                  for w0 in range(0, NW, WB):
                      nb = min(WB, NW - w0)
                      sl = slice(w0, w0 + nb)
                      h1 = blk_pool.tile([128, WB, F], f32, tag="h1")
                      nc.vector.tensor_tensor(
                          h1[:, 0:nb], num_all[:, sl, 0:F],
                          denr[:, sl].unsqueeze(2)
                          .broadcast_to([128, nb, F]),
                          ALU.mult)
                      nc.gpsimd.tensor_tensor(
                          h1[:, 0:nb], h1[:, 0:nb],
                          b_sb[:].unsqueeze(1).broadcast_to([128, nb, F]),
                          ALU.add)
                      mn = blk_pool.tile([128, WB, F], f32, tag="mn")
                      nc.vector.tensor_scalar_min(mn[:, 0:nb], h1[:, 0:nb],
                                                  0.0)
                      ex = blk_pool.tile([128, WB, F], f32, tag="ex")
                      nc.scalar.activation(ex[:, 0:nb], mn[:, 0:nb], AF.Exp)
                      nc.vector.scalar_tensor_tensor(
                          h_all[:, sl], ex[:, 0:nb], -1.0, h1[:, 0:nb],
                          ALU.add, ALU.max)

                  if debug_dump and k == 0:
                      nc.sync.dma_start(
                          dump_lh.ap().rearrange("t p f -> p t f"),
                          lh_stage[:])
                      nc.sync.dma_start(dump_pc.ap()[:], pc_stage[:])
                      nc.sync.dma_start(dump_adt.ap()[:], adt_all[:])
                      nc.sync.dma_start(dump_adrep.ap()[:], ad_rep[:])
                      nc.sync.dma_start(dump_num.ap()[:], num_all[:])
                      nc.sync.dma_start(dump_cc.ap()[:],
                                        cc_outs[0].ap()[0, 0:256])
                  if k < 2:
                      # transpose h tiles into hT for next layer
                      for j in range(NW):
                          pt = pst_pool.tile([128, 128], bf16, tag="ps_tr")
                          nc.tensor.transpose(pt[:], h_all[:, j], identity[:])
                          nc.vector.tensor_copy(
                              hT[:, 128 * j:128 * (j + 1)], pt[:])
                      if debug_dump and k == 0:
                          nc.sync.dma_start(dump_hT.ap()[:], hT[:])
                  else:
                      ps_pl = psp_pool.tile([G, F], f32, tag="ps_pl")
                      for j in range(NW):
                          nc.tensor.matmul(ps_pl[:], phot_sb[:, j],
                                           h_all[:, j], start=(j == 0),
                                           stop=(j == NW - 1))
                      pl_sb = post_pool.tile([G, F], f32, tag="pl_sb")
                      nc.vector.tensor_copy(pl_sb[:], ps_pl[:])
                      nc.sync.dma_start(pool_out.ap()[:], pl_sb[:])

    nc.compile()
    return nc


# ---------------- entry point ----------------

LAST_EXEC_NS = None
LAST_SCHED = None


def kernel(x, edge_index, batch,
           W1, a_src1, a_dst1, b1,
           W2, a_src2, a_dst2, b2,
           W3, a_src3, a_dst3, b3):
    global LAST_EXEC_NS, LAST_SCHED
    Ws = [np.asarray(W1, np.float32), np.asarray(W2, np.float32),
          np.asarray(W3, np.float32)]
    asrcs = [np.asarray(a_src1, np.float32), np.asarray(a_src2, np.float32),
             np.asarray(a_src3, np.float32)]
    adsts = [np.asarray(a_dst1, np.float32), np.asarray(a_dst2, np.float32),
             np.asarray(a_dst3, np.float32)]
    bs = [np.asarray(b1, np.float32), np.asarray(b2, np.float32),
          np.asarray(b3, np.float32)]

    in_maps, sched, per_core, counts = _prep_inputs(
        np.asarray(x), np.asarray(edge_index), np.asarray(batch),
        Ws, asrcs, adsts, bs)
    LAST_SCHED = sched

    from concourse.bass_utils import run_bass_kernel_spmd
    nc = _build_program(sched)
    res = run_bass_kernel_spmd(nc, in_maps, core_ids=list(range(NCORES)))
    LAST_EXEC_NS = res.exec_time_ns
    total = np.zeros((G, F), np.float32)
    for r in range(NCORES):
        total += res.results[r]["pool_part"]
    out = total / np.maximum(counts, 1.0)[:, None]
    return out.astype(np.float32)
